# revision 1
# baseline (speedup 1.0000x reference)
"""GaussianBlur2d Trainium2 kernel: 13x13 separable gaussian blur, reflect pad.

Input : x [32, 1, 1024, 1024] f32, kernel [1, 1, 13, 13] f32 (rank-1 separable).
Output: [32, 1, 1024, 1024] f32.

Strategy (pure data parallel, 4 images per core on 8 cores):
  The 2D conv is factored (SVD rank-1) into a vertical and a horizontal
  13-tap pass. Each pass runs on the TensorEngine as banded matmuls with
  the IMAGE TILE as the stationary operand:

     out[m=col, n=out_row] = sum_k  Xseg[k=row, m=col] * B[k=row, n=out_row]

  which both applies the 13-tap band (B) along the contraction (row) dim
  and transposes the tile - so after pass 1 the intermediate T1^T has
  partition=col, which is exactly the contraction layout pass 2 needs.
  Reflect padding is folded into the band matrices of the edge segments.
  PSUM windows are [128, 512] (one f32 bank); each 128-row segment
  contributes one banded matmul per window it overlaps (10 MMs per
  column-group: 8 mains + 2 boundary spills).
"""
import numpy as np

import concourse.bacc as bacc
import concourse.mybir as mybir
import concourse.tile as tile
from concourse import bass_utils

F32 = mybir.dt.float32

H = 1024          # image rows/cols
SEG = 128         # contraction segment (partition dim)
NSEG = H // SEG   # 8
WIN = 512         # psum window (one f32 bank)
NWIN = H // WIN   # 2
KS = 13
HALF = KS // 2
N_CORES = 8
IMGS_PER_CORE = 4
BAND_COLS = 1108  # per pass


def _reflect(r):
    if r < 0:
        return -r
    if r > H - 1:
        return 2 * (H - 1) - r
    return r


def _decompose_kernel(k2d):
    k = np.asarray(k2d, dtype=np.float64).reshape(KS, KS)
    u, s, vh = np.linalg.svd(k)
    gv = u[:, 0] * np.sqrt(s[0])
    gh = vh[0, :] * np.sqrt(s[0])
    if gv.sum() < 0:
        gv, gh = -gv, -gh
    return gv, gh


def _seg_ranges(s):
    lo = max(SEG * s - HALF, 0)
    hi = min(SEG * s + SEG - 1 + HALF, H - 1)
    pieces = []
    for w in range(lo // WIN, hi // WIN + 1):
        a = max(lo, WIN * w)
        b = min(hi, WIN * w + WIN - 1)
        pieces.append((w, a - WIN * w, b - WIN * w + 1, a))
    return pieces


def _plan_mms():
    """Per-column-group MM plan; no two consecutive MMs share a stationary."""
    mains, spills = [], []
    for s in range(NSEG):
        for (w, n0, n1, g0) in _seg_ranges(s):
            if w == s // (NSEG // NWIN):
                mains.append((s, w, n0, n1, g0))
            else:
                spills.append((s, w, n0, n1, g0))
    order = []
    pend = list(spills)
    prev = None
    for m in mains:
        order.append(m)
        prev = m[0]
        rest = []
        for sp in pend:
            if sp[0] <= m[0] and sp[0] != prev:
                order.append(sp)
                prev = sp[0]
            else:
                rest.append(sp)
        pend = rest
    order.extend(pend)
    for i in range(1, len(order)):
        assert order[i][0] != order[i - 1][0], "same-stationary adjacency"
    # annotate first/last per window and band column offsets
    cols = 0
    seen = set()
    plan = []
    for (s, w, n0, n1, g0) in order:
        width = n1 - n0
        plan.append([s, w, n0, n1, g0, cols, width, w not in seen, False])
        seen.add(w)
        cols += width
    assert cols == BAND_COLS
    last = {}
    for i, p in enumerate(plan):
        last[p[1]] = i
    for i in last.values():
        plan[i][8] = True
    return plan


_PLAN = _plan_mms()


def _build_bands(g):
    """Concatenated band matrices [128, BAND_COLS] f32 for one pass."""
    out = np.zeros((SEG, BAND_COLS), dtype=np.float64)
    for (s, w, n0, n1, g0, off, width, first, last) in _PLAN:
        for n in range(width):
            for t in range(KS):
                rr = _reflect(g0 + n - HALF + t)
                if SEG * s <= rr < SEG * s + SEG:
                    out[rr - SEG * s, off + n] += g[t]
    return out.astype(np.float32)


def _build_program():
    nc = bacc.Bacc("TRN2", target_bir_lowering=False, debug=False)
    x = nc.dram_tensor("x", [IMGS_PER_CORE, H, H], F32, kind="ExternalInput")
    bands = nc.dram_tensor("bands", [SEG, 2 * BAND_COLS], F32, kind="ExternalInput")
    y = nc.dram_tensor("y", [IMGS_PER_CORE, H, H], F32, kind="ExternalOutput")

    with tile.TileContext(nc) as tc:
        with (
            tc.tile_pool(name="xp", bufs=2) as xp,
            tc.tile_pool(name="t1p", bufs=1) as t1p,
            tc.tile_pool(name="op", bufs=2) as op,
            tc.tile_pool(name="bp", bufs=1) as bp,
            tc.tile_pool(name="pv", bufs=3, space="PSUM") as pvp,
            tc.tile_pool(name="ph", bufs=3, space="PSUM") as php,
        ):
            bt = bp.tile([SEG, 2 * BAND_COLS], F32, tag="bands")
            nc.sync.dma_start(bt[:], bands[:])

            for b in range(IMGS_PER_CORE):
                xt = xp.tile([SEG, NSEG * H], F32, name="xt", tag="x")
                nc.sync.dma_start(
                    xt[:].rearrange("p (s c) -> p s c", s=NSEG),
                    x[b].rearrange("(s p) c -> p s c", p=SEG),
                )
                t1 = t1p.tile([SEG, NSEG * H], F32, name="t1", tag="t1")
                # pass 1: vertical taps; output T1^T tiles [col, row]
                for cb in range(NSEG):
                    psums = {}
                    for (s, w, n0, n1, g0, off, width, first, last) in _PLAN:
                        if w not in psums:
                            psums[w] = pvp.tile([SEG, WIN], F32, name="psv", tag="psv")
                        nc.tensor.matmul(
                            psums[w][:, n0:n1],
                            xt[:, s * H + cb * SEG: s * H + cb * SEG + SEG],
                            bt[:, off: off + width],
                            start=first, stop=last,
                        )
                    for w in range(NWIN):
                        nc.vector.tensor_copy(
                            t1[:, cb * H + w * WIN: cb * H + (w + 1) * WIN],
                            psums[w][:],
                        )
                ot = op.tile([SEG, NSEG * H], F32, name="ot", tag="o")
                # pass 2: horizontal taps on T1^T; output Y tiles [row, col]
                for j in range(NSEG):
                    psums = {}
                    for (s, w, n0, n1, g0, off, width, first, last) in _PLAN:
                        if w not in psums:
                            psums[w] = php.tile([SEG, WIN], F32, name="psh", tag="psh")
                        nc.tensor.matmul(
                            psums[w][:, n0:n1],
                            t1[:, s * H + j * SEG: s * H + j * SEG + SEG],
                            bt[:, BAND_COLS + off: BAND_COLS + off + width],
                            start=first, stop=last,
                        )
                    for w in range(NWIN):
                        nc.scalar.copy(
                            ot[:, j * H + w * WIN: j * H + (w + 1) * WIN],
                            psums[w][:],
                        )
                nc.sync.dma_start(
                    y[b].rearrange("(s p) c -> p s c", p=SEG),
                    ot[:].rearrange("p (s c) -> p s c", s=NSEG),
                )
    nc.compile()
    return nc


_NC_CACHE = None


def _get_program():
    global _NC_CACHE
    if _NC_CACHE is None:
        _NC_CACHE = _build_program()
    return _NC_CACHE


def run(x, kernel, trace=False, tmpdir=None):
    """Full-input entry. Returns (y, BassKernelResults)."""
    x = np.ascontiguousarray(np.asarray(x, dtype=np.float32).reshape(32, H, H))
    gv, gh = _decompose_kernel(kernel)
    bands = np.concatenate([_build_bands(gv), _build_bands(gh)], axis=1)
    nc = _get_program()
    in_maps = [
        {"x": x[c * IMGS_PER_CORE:(c + 1) * IMGS_PER_CORE], "bands": bands}
        for c in range(N_CORES)
    ]
    res = bass_utils.run_bass_kernel_spmd(
        nc, in_maps, core_ids=list(range(N_CORES)), trace=trace, tmpdir=tmpdir)
    y = np.concatenate([res.results[c]["y"] for c in range(N_CORES)], axis=0)
    return y.reshape(32, 1, H, H), res


def kernel(x, kernel):
    y, _ = run(x, kernel, trace=False)
    return y


# revision 3
# speedup vs baseline: 1.0864x; 1.0864x over previous
"""GaussianBlur2d Trainium2 kernel: 13x13 separable gaussian blur, reflect pad.

Input : x [32, 1, 1024, 1024] f32, kernel [1, 1, 13, 13] f32 (rank-1 separable).
Output: [32, 1, 1024, 1024] f32.

Strategy (pure data parallel, 4 images per core on 8 cores):
  The 2D conv is factored (SVD rank-1) into a vertical and a horizontal
  13-tap pass. Each pass runs on the TensorEngine as banded matmuls with
  the IMAGE TILE as the stationary operand:

     out[m=col, n=out_row] = sum_k  Xseg[k=row, m=col] * B[k=row, n=out_row]

  which both applies the 13-tap band (B) along the contraction (row) dim
  and transposes the tile - so after pass 1 the intermediate T1^T has
  partition=col, which is exactly the contraction layout pass 2 needs.
  Reflect padding is folded into the band matrices of the edge segments.
  PSUM windows are [128, 512] (one f32 bank); each 128-row segment
  contributes one banded matmul per window it overlaps (10 MMs per
  column-group: 8 mains + 2 boundary spills).
"""
import numpy as np

import concourse.bacc as bacc
import concourse.mybir as mybir
import concourse.tile as tile
from concourse import bass_utils

F32 = mybir.dt.float32

H = 1024          # image rows/cols
SEG = 128         # contraction segment (partition dim)
NSEG = H // SEG   # 8
WIN = 512         # psum window (one f32 bank)
NWIN = H // WIN   # 2
KS = 13
HALF = KS // 2
N_CORES = 8
IMGS_PER_CORE = 4
BAND_COLS = 1108  # per pass


def _reflect(r):
    if r < 0:
        return -r
    if r > H - 1:
        return 2 * (H - 1) - r
    return r


def _decompose_kernel(k2d):
    k = np.asarray(k2d, dtype=np.float64).reshape(KS, KS)
    u, s, vh = np.linalg.svd(k)
    gv = u[:, 0] * np.sqrt(s[0])
    gh = vh[0, :] * np.sqrt(s[0])
    if gv.sum() < 0:
        gv, gh = -gv, -gh
    return gv, gh


def _seg_ranges(s):
    lo = max(SEG * s - HALF, 0)
    hi = min(SEG * s + SEG - 1 + HALF, H - 1)
    pieces = []
    for w in range(lo // WIN, hi // WIN + 1):
        a = max(lo, WIN * w)
        b = min(hi, WIN * w + WIN - 1)
        pieces.append((w, a - WIN * w, b - WIN * w + 1, a))
    return pieces


def _plan_mms():
    """Per-column-group MM plan; no two consecutive MMs share a stationary."""
    mains, spills = [], []
    for s in range(NSEG):
        for (w, n0, n1, g0) in _seg_ranges(s):
            if w == s // (NSEG // NWIN):
                mains.append((s, w, n0, n1, g0))
            else:
                spills.append((s, w, n0, n1, g0))
    # emit mains in seg order; drop each spill in as soon as it can sit
    # between two mains (spills are tiny, LDW-bound — never adjacent to
    # each other or to their own seg's main so the weight load can hide)
    order = []
    pend = list(spills)
    last_was_spill = False
    for m in mains:
        order.append(m)
        last_was_spill = False
        for sp in list(pend):
            if sp[0] <= m[0] and sp[0] != order[-1][0] and not last_was_spill:
                order.append(sp)
                pend.remove(sp)
                last_was_spill = True
    order.extend(pend)
    for i in range(1, len(order)):
        assert order[i][0] != order[i - 1][0], "same-stationary adjacency"
    # annotate first/last per window and band column offsets
    cols = 0
    seen = set()
    plan = []
    for (s, w, n0, n1, g0) in order:
        width = n1 - n0
        plan.append([s, w, n0, n1, g0, cols, width, w not in seen, False])
        seen.add(w)
        cols += width
    assert cols == BAND_COLS
    last = {}
    for i, p in enumerate(plan):
        last[p[1]] = i
    for i in last.values():
        plan[i][8] = True
    return plan


_PLAN = _plan_mms()


def _build_bands(g):
    """Concatenated band matrices [128, BAND_COLS] f32 for one pass."""
    out = np.zeros((SEG, BAND_COLS), dtype=np.float64)
    for (s, w, n0, n1, g0, off, width, first, last) in _PLAN:
        for n in range(width):
            for t in range(KS):
                rr = _reflect(g0 + n - HALF + t)
                if SEG * s <= rr < SEG * s + SEG:
                    out[rr - SEG * s, off + n] += g[t]
    return out.astype(np.float32)


def _build_program():
    nc = bacc.Bacc("TRN2", target_bir_lowering=False, debug=False)
    x = nc.dram_tensor("x", [IMGS_PER_CORE, H, H], F32, kind="ExternalInput")
    bands = nc.dram_tensor("bands", [SEG, 2 * BAND_COLS], F32, kind="ExternalInput")
    y = nc.dram_tensor("y", [IMGS_PER_CORE, H, H], F32, kind="ExternalOutput")

    with tile.TileContext(nc) as tc:
        with (
            tc.tile_pool(name="xp", bufs=2) as xp,
            tc.tile_pool(name="t1p", bufs=1) as t1p,
            tc.tile_pool(name="op", bufs=2) as op,
            tc.tile_pool(name="bp", bufs=1) as bp,
            tc.tile_pool(name="pv", bufs=3, space="PSUM") as pvp,
            tc.tile_pool(name="ph", bufs=3, space="PSUM") as php,
        ):
            bt = bp.tile([SEG, 2 * BAND_COLS], F32, tag="bands")
            nc.sync.dma_start(bt[:], bands[:])

            for b in range(IMGS_PER_CORE):
                # per-segment input tiles: pass 1 can start after the first
                # segment lands instead of waiting for the whole image
                xts = []
                for s in range(NSEG):
                    xs = xp.tile([SEG, H], F32, name=f"xt{s}", tag=f"x{s}")
                    nc.sync.dma_start(xs[:], x[b, s * SEG:(s + 1) * SEG, :])
                    xts.append(xs)
                t1 = t1p.tile([SEG, NSEG * H], F32, name="t1", tag="t1")
                # pass 1: vertical taps; output T1^T tiles [col, row]
                for cb in range(NSEG):
                    psums = {}
                    for (s, w, n0, n1, g0, off, width, first, last) in _PLAN:
                        if w not in psums:
                            psums[w] = pvp.tile([SEG, WIN], F32, name="psv", tag="psv")
                        nc.tensor.matmul(
                            psums[w][:, n0:n1],
                            xts[s][:, cb * SEG: cb * SEG + SEG],
                            bt[:, off: off + width],
                            start=first, stop=last,
                        )
                    for w in range(NWIN):
                        nc.vector.tensor_copy(
                            t1[:, cb * H + w * WIN: cb * H + (w + 1) * WIN],
                            psums[w][:],
                        )
                # pass 2: horizontal taps on T1^T; output Y tiles [row, col];
                # per-row-block output tiles DMA out as soon as they complete
                for j in range(NSEG):
                    psums = {}
                    for (s, w, n0, n1, g0, off, width, first, last) in _PLAN:
                        if w not in psums:
                            psums[w] = php.tile([SEG, WIN], F32, name="psh", tag="psh")
                        nc.tensor.matmul(
                            psums[w][:, n0:n1],
                            t1[:, s * H + j * SEG: j * SEG + s * H + SEG],
                            bt[:, BAND_COLS + off: BAND_COLS + off + width],
                            start=first, stop=last,
                        )
                    oj = op.tile([SEG, H], F32, name=f"ot{j}", tag=f"o{j % 4}")
                    for w in range(NWIN):
                        nc.scalar.copy(
                            oj[:, w * WIN:(w + 1) * WIN],
                            psums[w][:],
                        )
                    nc.sync.dma_start(y[b, j * SEG:(j + 1) * SEG, :], oj[:])
    nc.compile()
    return nc


_NC_CACHE = None


def _get_program():
    global _NC_CACHE
    if _NC_CACHE is None:
        _NC_CACHE = _build_program()
    return _NC_CACHE


def run(x, kernel, trace=False, tmpdir=None):
    """Full-input entry. Returns (y, BassKernelResults)."""
    x = np.ascontiguousarray(np.asarray(x, dtype=np.float32).reshape(32, H, H))
    gv, gh = _decompose_kernel(kernel)
    bands = np.concatenate([_build_bands(gv), _build_bands(gh)], axis=1)
    nc = _get_program()
    in_maps = [
        {"x": x[c * IMGS_PER_CORE:(c + 1) * IMGS_PER_CORE], "bands": bands}
        for c in range(N_CORES)
    ]
    res = bass_utils.run_bass_kernel_spmd(
        nc, in_maps, core_ids=list(range(N_CORES)), trace=trace, tmpdir=tmpdir)
    y = np.concatenate([res.results[c]["y"] for c in range(N_CORES)], axis=0)
    return y.reshape(32, 1, H, H), res


def kernel(x, kernel):
    y, _ = run(x, kernel, trace=False)
    return y
